# revision 18
# baseline (speedup 1.0000x reference)
"""CQAttention Trainium2 kernel.

Math (per batch b):
  S = (C*w3) @ Q^T + (C@w1)[:,None] + (Q@w2)[None,:] (+bias, dropped: softmax-invariant)
  Sq = softmax over q of qmask-masked S ; Sc = softmax over c of cmask-masked S
  A = Sq@Q ; Bm = Sq @ (Sc^T @ C) ; out = [C | A | C*A | C*Bm]

Device algorithm (no max-subtraction: |S| < 1 so exp is safe; masks become
additive -1e30 terms, i.e. multiplicative exp factors that either cancel in
the normalized ratios or are folded into operands):
  ST   = (Q*w3) @ C^T                       [q, c]   (PE, fp32r)
  E_q  = exp(ST + (rq + qneg)[q])           [q, c]   row-masked
  X    = exp(ST)                            [q, c]
  Cg   = [C|1] * exp(rc + cneg)[c]          [c, d+2] col-mask folded into C
  T1s  = (X^T @ Cg) normalized              [q, d]   == Sc^T @ C
  psA  = E_q^T @ [Q|1]                      [c, d+2] unnormalized A | rowsum
  psB  = E_q^T @ T1s                        [c, d]   unnormalized Bm
  A = psA * rr ; CA = C * A ; CBm = C * psB * rr      (rr = 1/rowsum)

Sharding: data-parallel over batch, 4 batches per core on 8 cores.
"""

import os

import numpy as np

NEG_INF = -1e30
B_FULL, LC, LQ, D = 32, 1024, 128, 256
N_CORES = 8
NB = B_FULL // N_CORES  # batches per core
KC = LC // 128  # c-tiles per batch (8)

_CACHE = {}


def _build_nc():
    import concourse.bacc as bacc
    import concourse.mybir as mybir
    from concourse import tile
    from concourse.masks import make_identity

    fp32 = mybir.dt.float32
    MULT = mybir.AluOpType.mult
    ADD = mybir.AluOpType.add
    EXP = mybir.ActivationFunctionType.Exp

    nc = bacc.Bacc("TRN2", target_bir_lowering=False, debug=False)

    use_r = os.environ.get("FP32R", "1") != "0"
    fp32r = mybir.dt.float32r
    mmdt = fp32r if use_r else fp32

    C_d = nc.dram_tensor("C", [NB, LC, D], fp32, kind="ExternalInput")
    Q_d = nc.dram_tensor("Q", [NB, LQ, D], fp32, kind="ExternalInput")
    cneg_d = nc.dram_tensor("cneg", [NB, 128, KC], fp32, kind="ExternalInput")
    qneg_d = nc.dram_tensor("qneg", [NB, 128, 1], fp32, kind="ExternalInput")
    w_d = nc.dram_tensor("w_pk", [128, 6], fp32, kind="ExternalInput")
    w2bc_d = nc.dram_tensor("w2bc", [128, D], fp32, kind="ExternalInput")
    out_d = nc.dram_tensor("out", [NB, LC, 4 * D], fp32, kind="ExternalOutput")

    with tile.TileContext(nc) as tc:
        with (
            tc.tile_pool(name="const", bufs=1) as const,
            tc.tile_pool(name="cpool", bufs=NB) as p_c,
            tc.tile_pool(name="cgpool", bufs=3) as p_cg,
            tc.tile_pool(name="qpool", bufs=NB) as p_q,
            tc.tile_pool(name="mpool", bufs=NB) as p_m,
            tc.tile_pool(name="ctpool", bufs=3) as p_ct,
            tc.tile_pool(name="qtpool", bufs=2) as p_qt,
            tc.tile_pool(name="epool", bufs=3) as p_e,
            tc.tile_pool(name="opool", bufs=3) as p_o,
            tc.tile_pool(name="smpool", bufs=4) as p_sm,
            tc.tile_pool(name="pst", bufs=3, space="PSUM") as ps_t,
            tc.tile_pool(name="psst", bufs=1, space="PSUM") as ps_st,
            tc.tile_pool(name="pst1", bufs=1, space="PSUM") as ps_t1,
            tc.tile_pool(name="psa", bufs=2, space="PSUM") as ps_a,
            tc.tile_pool(name="psb", bufs=1, space="PSUM") as ps_b,
        ):
            ident = const.tile([128, 128], fp32)
            make_identity(nc, ident)
            w_sb = const.tile([128, 6], fp32)
            nc.sync.dma_start(w_sb, w_d.ap())
            w2bc = const.tile([128, D], fp32)
            nc.sync.dma_start(w2bc, w2bc_d.ap())
            # duplicated-column w1 (fp32r, even-N rhs for the rc matmuls)
            w1r2 = const.tile([128, 2, 2], mmdt, tag="w1r2")
            for dk in range(2):
                for j in range(2):
                    nc.vector.tensor_copy(w1r2[:, dk, j : j + 1], w_sb[:, dk : dk + 1])

            # ---- hoisted input loads for all batches ----
            C1s, Q1s, cnegs, qnegs = [], [], [], []
            for b in range(NB):
                C1 = p_c.tile([128, KC, D + 2], fp32, tag="c")
                nc.vector.memset(C1[:, :, D : D + 2], 1.0)
                nc.sync.dma_start(
                    C1[:, :, 0:D], C_d.ap()[b].rearrange("(k p) d -> p k d", p=128)
                )
                Q1 = p_q.tile([128, D + 2], fp32, tag="q")
                nc.vector.memset(Q1[:, D : D + 2], 1.0)
                nc.sync.dma_start(Q1[:, 0:D], Q_d.ap()[b])
                cneg = p_m.tile([128, KC], fp32, tag="cneg")
                nc.sync.dma_start(cneg, cneg_d.ap()[b])
                qneg = p_m.tile([128, 1], fp32, tag="qneg")
                nc.sync.dma_start(qneg, qneg_d.ap()[b])
                # C segment of the output goes straight from SBUF.
                nc.sync.dma_start(
                    out_d.ap()[b, :, 0:D].rearrange("(k p) d -> p k d", p=128),
                    C1[:, :, 0:D],
                )
                C1s.append(C1)
                Q1s.append(Q1)
                cnegs.append(cneg)
                qnegs.append(qneg)

            for b in range(NB):
                C1, Q1, cneg, qneg = C1s[b], Q1s[b], cnegs[b], qnegs[b]

                # rounded [Q | 1] rhs
                if use_r:
                    Q1r = p_q.tile([128, D + 2], fp32r, tag="qr")
                    nc.vector.tensor_copy(Q1r, Q1)
                else:
                    Q1r = Q1

                # ---- rq = Q@w2 (gpsimd product + DVE reduce) ----
                scr = p_sm.tile([128, D], fp32, tag="ttrs")
                nc.gpsimd.tensor_mul(scr, Q1[:, 0:D], w2bc)
                rq = p_sm.tile([128, 1], fp32, tag="rq")
                nc.vector.tensor_reduce(rq, scr, mybir.AxisListType.X, ADD)
                bias_q = p_sm.tile([128, 1], fp32, tag="biasq")
                nc.vector.tensor_add(bias_q, rq, qneg)

                # ---- QT3 = (Q^T) * w3 per d-chunk ----
                QT3 = p_qt.tile([128, 2, 128], mmdt, tag="qtw3")
                for dk in range(2):
                    pt = ps_t.tile([128, 512], fp32, tag="pt")
                    nc.tensor.transpose(
                        pt[:, 0:128], Q1[:, dk * 128 : (dk + 1) * 128], ident
                    )
                    nc.vector.tensor_scalar_mul(
                        QT3[:, dk], pt[:, 0:128], w_sb[:, 4 + dk : 5 + dk]
                    )

                # ---- CT (transpose C): 4 transposes per PSUM bank, 1 copy ----
                CT = p_ct.tile([128, 2, LC], mmdt, tag="ct")
                for dk in range(2):
                    for h in range(2):
                        pt = ps_t.tile([128, 512], fp32, tag="pt")
                        for j in range(4):
                            k = h * 4 + j
                            nc.tensor.transpose(
                                pt[:, j * 128 : (j + 1) * 128],
                                C1[:, k, dk * 128 : (dk + 1) * 128],
                                ident,
                            )
                        dst = CT[:, dk, h * 512 : (h + 1) * 512]
                        if (dk * 2 + h) % 2 == 0:
                            nc.scalar.copy(dst, pt)
                        else:
                            nc.vector.tensor_copy(dst, pt)

                # ---- rc = C@w1 on PE (tiny fp32r matmuls on CT) ----
                rc_ps = ps_t.tile([128, 2 * KC], fp32, tag="pt")
                for k in range(KC):
                    for dk in range(2):
                        nc.tensor.matmul(
                            rc_ps[:, 2 * k : 2 * k + 2],
                            CT[:, dk, k * 128 : (k + 1) * 128],
                            w1r2[:, dk],
                            start=(dk == 0),
                            stop=(dk == 1),
                        )
                rc_cneg = p_sm.tile([128, KC], fp32, tag="rccneg")
                nc.vector.tensor_add(
                    rc_cneg,
                    rc_ps.rearrange("p (k two) -> p k two", two=2)[:, :, 0],
                    cneg,
                )
                gexp = p_sm.tile([128, KC], fp32, tag="gexp")
                nc.scalar.activation(gexp, rc_cneg, EXP)

                # ---- Cg = [C|1] * exp(rc+cneg): col-mask folded into rhs ----
                Cg = p_cg.tile([128, KC, D + 2], mmdt, tag="cg")
                for k in range(KC):
                    nc.vector.tensor_scalar_mul(Cg[:, k], C1[:, k], gexp[:, k : k + 1])

                # ---- main matmul ST = (Q*w3) @ C^T, then E_q / X ----
                E_q = p_e.tile([128, LC], mmdt, tag="eq")
                X = p_e.tile([128, LC], fp32, tag="x")
                for h in range(2):
                    st = ps_st.tile([128, 512], fp32, tag="st")
                    for dk in range(2):
                        nc.tensor.matmul(
                            st,
                            QT3[:, dk],
                            CT[:, dk, h * 512 : (h + 1) * 512],
                            start=(dk == 0),
                            stop=(dk == 1),
                        )
                    nc.scalar.activation(
                        E_q[:, h * 512 : (h + 1) * 512], st, EXP, bias=bias_q
                    )
                    nc.scalar.activation(X[:, h * 512 : (h + 1) * 512], st, EXP)

                # ---- XT = X^T (raw; mask/rc factors live in Cg) ----
                XT = p_e.tile([128, KC, 128], mmdt, tag="xt")
                XTflat = XT.rearrange("p k q -> p (k q)")
                for h in range(2):
                    pt = ps_t.tile([128, 512], fp32, tag="pt")
                    for j in range(4):
                        k = h * 4 + j
                        nc.tensor.transpose(
                            pt[:, j * 128 : (j + 1) * 128],
                            X[:, k * 128 : (k + 1) * 128],
                            ident,
                        )
                    dst = XTflat[:, h * 512 : (h + 1) * 512]
                    if h % 2 == 0:
                        nc.scalar.copy(dst, pt)
                    else:
                        nc.vector.tensor_copy(dst, pt)

                # ---- T1s = (Sc^T C) = (X^T @ Cg) normalized ----
                t1 = ps_t1.tile([128, D + 2], fp32, tag="t1")
                for k in range(KC):
                    nc.tensor.matmul(
                        t1,
                        XT[:, k],
                        Cg[:, k],
                        start=(k == 0),
                        stop=(k == KC - 1),
                    )
                recipT = p_sm.tile([128, 1], fp32, tag="recipT")
                nc.vector.reciprocal(recipT, t1[:, D : D + 1])
                T1s = p_sm.tile([128, D], mmdt, tag="t1s")
                nc.vector.tensor_scalar_mul(T1s, t1[:, 0:D], recipT)

                # ---- per c-tile: A / CA / CBm ----
                for k in range(KC):
                    eq_k = E_q[:, k * 128 : (k + 1) * 128]
                    psA = ps_a.tile([128, D + 2], fp32, tag="psa")
                    nc.tensor.matmul(psA, eq_k, Q1r[:], start=True, stop=True)
                    psB = ps_b.tile([128, D], fp32, tag="psb")
                    nc.tensor.matmul(psB, eq_k, T1s[:], start=True, stop=True)

                    rr = p_sm.tile([128, 1], fp32, tag="rr")
                    nc.vector.reciprocal(rr, psA[:, D : D + 1])

                    osb = p_o.tile([128, 3 * D], fp32, tag="osb")
                    # A = psA * rr  (ACT, per-partition scale)
                    nc.scalar.mul(osb[:, 0:D], psA[:, 0:D], rr)
                    # CA = C * A  (GPSIMD, reads the extracted A)
                    nc.gpsimd.tensor_mul(osb[:, D : 2 * D], C1[:, k, 0:D], osb[:, 0:D])
                    # CBm = (psB * rr) * C  (DVE fused)
                    nc.vector.scalar_tensor_tensor(
                        osb[:, 2 * D : 3 * D], psB, rr, C1[:, k, 0:D], MULT, MULT
                    )
                    nc.sync.dma_start(
                        out_d.ap()[b, k * 128 : (k + 1) * 128, D : 4 * D], osb
                    )

    nc.compile()
    return nc


def _get_nc():
    if "nc" not in _CACHE:
        _CACHE["nc"] = _build_nc()
    return _CACHE["nc"]


def _make_in_maps(C, Q, cmask, qmask, Wo_w):
    C = np.ascontiguousarray(C, dtype=np.float32)
    Q = np.ascontiguousarray(Q, dtype=np.float32)
    cneg = ((1.0 - cmask.astype(np.float32)) * NEG_INF).astype(np.float32)
    qneg = ((1.0 - qmask.astype(np.float32)) * NEG_INF).astype(np.float32)
    cneg = np.ascontiguousarray(cneg.reshape(B_FULL, KC, 128).transpose(0, 2, 1))
    qneg = np.ascontiguousarray(qneg.reshape(B_FULL, 128, 1))
    Wo_w = Wo_w.astype(np.float32)
    w_pk = np.ascontiguousarray(Wo_w.reshape(6, 128).T)
    w2bc = np.ascontiguousarray(np.broadcast_to(Wo_w[D : 2 * D], (128, D)))
    in_maps = []
    for i in range(N_CORES):
        sl = slice(i * NB, (i + 1) * NB)
        in_maps.append(
            {
                "C": np.ascontiguousarray(C[sl]),
                "Q": np.ascontiguousarray(Q[sl]),
                "cneg": np.ascontiguousarray(cneg[sl]),
                "qneg": np.ascontiguousarray(qneg[sl]),
                "w_pk": w_pk,
                "w2bc": w2bc,
            }
        )
    return in_maps


def kernel(C, Q, cmask, qmask, Wo_w, Wo_b):
    from concourse.bass_utils import run_bass_kernel_spmd

    nc = _get_nc()
    in_maps = _make_in_maps(C, Q, cmask, qmask, Wo_w)
    res = run_bass_kernel_spmd(nc, in_maps, core_ids=list(range(N_CORES)))
    out = np.concatenate([res.results[i]["out"] for i in range(N_CORES)], axis=0)
    return out


# revision 19
# speedup vs baseline: 1.1465x; 1.1465x over previous
"""CQAttention Trainium2 kernel.

Math (per batch b):
  S = (C*w3) @ Q^T + (C@w1)[:,None] + (Q@w2)[None,:] (+bias, dropped: softmax-invariant)
  Sq = softmax over q of qmask-masked S ; Sc = softmax over c of cmask-masked S
  A = Sq@Q ; Bm = Sq @ (Sc^T @ C) ; out = [C | A | C*A | C*Bm]

Device algorithm (no max-subtraction: |S| < 1 so exp is safe; masks become
additive -1e30 terms, i.e. multiplicative exp factors that either cancel in
the normalized ratios or are folded into operands):
  ST   = (Q*w3) @ C^T                       [q, c]   (PE, fp32r)
  E_q  = exp(ST + (rq + qneg)[q])           [q, c]   row-masked
  X    = exp(ST)                            [q, c]
  Cg   = [C|1] * exp(rc + cneg)[c]          [c, d+2] col-mask folded into C
  T1s  = (X^T @ Cg) normalized              [q, d]   == Sc^T @ C
  psA  = E_q^T @ [Q|1]                      [c, d+2] unnormalized A | rowsum
  psB  = E_q^T @ T1s                        [c, d]   unnormalized Bm
  A = psA * rr ; CA = C * A ; CBm = C * psB * rr      (rr = 1/rowsum)

Sharding: data-parallel over batch, 4 batches per core on 8 cores.
"""

import os

import numpy as np

NEG_INF = -1e30
B_FULL, LC, LQ, D = 32, 1024, 128, 256
N_CORES = 8
NB = B_FULL // N_CORES  # batches per core
KC = LC // 128  # c-tiles per batch (8)

_CACHE = {}


def _build_nc():
    import concourse.bacc as bacc
    import concourse.mybir as mybir
    from concourse import tile
    from concourse.masks import make_identity

    fp32 = mybir.dt.float32
    MULT = mybir.AluOpType.mult
    ADD = mybir.AluOpType.add
    EXP = mybir.ActivationFunctionType.Exp

    nc = bacc.Bacc("TRN2", target_bir_lowering=False, debug=False)

    use_r = os.environ.get("FP32R", "1") != "0"
    fp32r = mybir.dt.float32r
    mmdt = fp32r if use_r else fp32

    C_d = nc.dram_tensor("C", [NB, LC, D], fp32, kind="ExternalInput")
    Q_d = nc.dram_tensor("Q", [NB, LQ, D], fp32, kind="ExternalInput")
    cneg_d = nc.dram_tensor("cneg", [NB, 128, KC], fp32, kind="ExternalInput")
    qneg_d = nc.dram_tensor("qneg", [NB, 128, 1], fp32, kind="ExternalInput")
    w_d = nc.dram_tensor("w_pk", [128, 6], fp32, kind="ExternalInput")
    w2bc_d = nc.dram_tensor("w2bc", [128, D], fp32, kind="ExternalInput")
    out_d = nc.dram_tensor("out", [NB, LC, 4 * D], fp32, kind="ExternalOutput")

    with tile.TileContext(nc) as tc:
        with (
            tc.tile_pool(name="const", bufs=1) as const,
            tc.tile_pool(name="cpool", bufs=NB) as p_c,
            tc.tile_pool(name="cgpool", bufs=2) as p_cg,
            tc.tile_pool(name="qpool", bufs=NB) as p_q,
            tc.tile_pool(name="mpool", bufs=NB) as p_m,
            tc.tile_pool(name="ctpool", bufs=2) as p_ct,
            tc.tile_pool(name="qtpool", bufs=2) as p_qt,
            tc.tile_pool(name="epool", bufs=2) as p_e,
            tc.tile_pool(name="opool", bufs=3) as p_o,
            tc.tile_pool(name="smpool", bufs=4) as p_sm,
            tc.tile_pool(name="pst", bufs=2, space="PSUM") as ps_t,
            tc.tile_pool(name="psst", bufs=2, space="PSUM") as ps_st,
            tc.tile_pool(name="pst1", bufs=1, space="PSUM") as ps_t1,
            tc.tile_pool(name="psa", bufs=2, space="PSUM") as ps_a,
            tc.tile_pool(name="psb", bufs=1, space="PSUM") as ps_b,
        ):
            ident = const.tile([128, 128], fp32)
            make_identity(nc, ident)
            w_sb = const.tile([128, 6], fp32)
            nc.sync.dma_start(w_sb, w_d.ap())
            w2bc = const.tile([128, D], fp32)
            nc.sync.dma_start(w2bc, w2bc_d.ap())
            # duplicated-column w1 (fp32r, even-N rhs for the rc matmuls)
            w1r2 = const.tile([128, 2, 2], mmdt, tag="w1r2")
            for dk in range(2):
                for j in range(2):
                    nc.vector.tensor_copy(w1r2[:, dk, j : j + 1], w_sb[:, dk : dk + 1])

            # ---- hoisted input loads for all batches ----
            C1s, Q1s, cnegs, qnegs = [], [], [], []
            for b in range(NB):
                C1 = p_c.tile([128, KC, D + 2], fp32, tag="c")
                nc.vector.memset(C1[:, :, D : D + 2], 1.0)
                nc.sync.dma_start(
                    C1[:, :, 0:D], C_d.ap()[b].rearrange("(k p) d -> p k d", p=128)
                )
                Q1 = p_q.tile([128, D + 2], fp32, tag="q")
                nc.vector.memset(Q1[:, D : D + 2], 1.0)
                nc.sync.dma_start(Q1[:, 0:D], Q_d.ap()[b])
                cneg = p_m.tile([128, KC], fp32, tag="cneg")
                nc.sync.dma_start(cneg, cneg_d.ap()[b])
                qneg = p_m.tile([128, 1], fp32, tag="qneg")
                nc.sync.dma_start(qneg, qneg_d.ap()[b])
                # C segment of the output goes straight from SBUF.
                nc.sync.dma_start(
                    out_d.ap()[b, :, 0:D].rearrange("(k p) d -> p k d", p=128),
                    C1[:, :, 0:D],
                )
                C1s.append(C1)
                Q1s.append(Q1)
                cnegs.append(cneg)
                qnegs.append(qneg)

            for b in range(NB):
                C1, Q1, cneg, qneg = C1s[b], Q1s[b], cnegs[b], qnegs[b]

                # rounded [Q | 1] rhs
                if use_r:
                    Q1r = p_q.tile([128, D + 2], fp32r, tag="qr")
                    nc.vector.tensor_copy(Q1r, Q1)
                else:
                    Q1r = Q1

                # ---- rq = Q@w2 (gpsimd product + DVE reduce) ----
                scr = p_sm.tile([128, D], fp32, tag="ttrs")
                nc.gpsimd.tensor_mul(scr, Q1[:, 0:D], w2bc)
                rq = p_sm.tile([128, 1], fp32, tag="rq")
                nc.vector.tensor_reduce(rq, scr, mybir.AxisListType.X, ADD)
                bias_q = p_sm.tile([128, 1], fp32, tag="biasq")
                nc.vector.tensor_add(bias_q, rq, qneg)

                # ---- QT3 = (Q^T) * w3 per d-chunk ----
                QT3 = p_qt.tile([128, 2, 128], mmdt, tag="qtw3")
                for dk in range(2):
                    pt = ps_t.tile([128, 512], fp32, tag="pt")
                    nc.tensor.transpose(
                        pt[:, 0:128], Q1[:, dk * 128 : (dk + 1) * 128], ident
                    )
                    nc.vector.tensor_scalar_mul(
                        QT3[:, dk], pt[:, 0:128], w_sb[:, 4 + dk : 5 + dk]
                    )

                # ---- CT (transpose C): 4 transposes per PSUM bank, 1 copy ----
                CT = p_ct.tile([128, 2, LC], mmdt, tag="ct")
                for dk in range(2):
                    for h in range(2):
                        pt = ps_t.tile([128, 512], fp32, tag="pt")
                        for j in range(4):
                            k = h * 4 + j
                            nc.tensor.transpose(
                                pt[:, j * 128 : (j + 1) * 128],
                                C1[:, k, dk * 128 : (dk + 1) * 128],
                                ident,
                            )
                        dst = CT[:, dk, h * 512 : (h + 1) * 512]
                        if (dk * 2 + h) % 2 == 0:
                            nc.scalar.copy(dst, pt)
                        else:
                            nc.vector.tensor_copy(dst, pt)

                # ---- rc = C@w1 on PE (tiny fp32r matmuls on CT) ----
                rc_ps = ps_t.tile([128, 2 * KC], fp32, tag="pt")
                for k in range(KC):
                    for dk in range(2):
                        nc.tensor.matmul(
                            rc_ps[:, 2 * k : 2 * k + 2],
                            CT[:, dk, k * 128 : (k + 1) * 128],
                            w1r2[:, dk],
                            start=(dk == 0),
                            stop=(dk == 1),
                        )
                rc_cneg = p_sm.tile([128, KC], fp32, tag="rccneg")
                nc.vector.tensor_add(
                    rc_cneg,
                    rc_ps.rearrange("p (k two) -> p k two", two=2)[:, :, 0],
                    cneg,
                )
                gexp = p_sm.tile([128, KC], fp32, tag="gexp")
                nc.scalar.activation(gexp, rc_cneg, EXP)

                # ---- Cg = [C|1] * exp(rc+cneg): col-mask folded into rhs ----
                Cg = p_cg.tile([128, KC, D + 2], mmdt, tag="cg")
                for k in range(KC):
                    nc.vector.tensor_scalar_mul(Cg[:, k], C1[:, k], gexp[:, k : k + 1])

                # ---- main matmul ST = (Q*w3) @ C^T, then E_q / X ----
                E_q = p_e.tile([128, LC], mmdt, tag="eq")
                X = p_e.tile([128, LC], fp32, tag="x")
                for h in range(2):
                    st = ps_st.tile([128, 512], fp32, tag="st")
                    for dk in range(2):
                        nc.tensor.matmul(
                            st,
                            QT3[:, dk],
                            CT[:, dk, h * 512 : (h + 1) * 512],
                            start=(dk == 0),
                            stop=(dk == 1),
                        )
                    nc.scalar.activation(
                        E_q[:, h * 512 : (h + 1) * 512], st, EXP, bias=bias_q
                    )
                    nc.scalar.activation(X[:, h * 512 : (h + 1) * 512], st, EXP)

                # ---- XT = X^T (raw; mask/rc factors live in Cg) ----
                XT = p_e.tile([128, KC, 128], mmdt, tag="xt")
                XTflat = XT.rearrange("p k q -> p (k q)")
                for h in range(2):
                    pt = ps_t.tile([128, 512], fp32, tag="pt")
                    for j in range(4):
                        k = h * 4 + j
                        nc.tensor.transpose(
                            pt[:, j * 128 : (j + 1) * 128],
                            X[:, k * 128 : (k + 1) * 128],
                            ident,
                        )
                    dst = XTflat[:, h * 512 : (h + 1) * 512]
                    if h % 2 == 0:
                        nc.scalar.copy(dst, pt)
                    else:
                        nc.vector.tensor_copy(dst, pt)

                # ---- T1s = (Sc^T C) = (X^T @ Cg) normalized ----
                t1 = ps_t1.tile([128, D + 2], fp32, tag="t1")
                for k in range(KC):
                    nc.tensor.matmul(
                        t1,
                        XT[:, k],
                        Cg[:, k],
                        start=(k == 0),
                        stop=(k == KC - 1),
                    )
                recipT = p_sm.tile([128, 1], fp32, tag="recipT")
                nc.vector.reciprocal(recipT, t1[:, D : D + 1])
                T1s = p_sm.tile([128, D], mmdt, tag="t1s")
                nc.vector.tensor_scalar_mul(T1s, t1[:, 0:D], recipT)

                # ---- per c-tile: A / CA / CBm (stores paired over 2 tiles) ----
                for k in range(KC):
                    kk = k % 2
                    if kk == 0:
                        osb = p_o.tile([128, 2, 3 * D], fp32, tag="osb")
                    eq_k = E_q[:, k * 128 : (k + 1) * 128]
                    psA = ps_a.tile([128, D + 2], fp32, tag="psa")
                    nc.tensor.matmul(psA, eq_k, Q1r[:], start=True, stop=True)
                    psB = ps_b.tile([128, D], fp32, tag="psb")
                    nc.tensor.matmul(psB, eq_k, T1s[:], start=True, stop=True)

                    rr = p_sm.tile([128, 1], fp32, tag="rr")
                    nc.vector.reciprocal(rr, psA[:, D : D + 1])

                    # A = psA * rr  (ACT, per-partition scale)
                    nc.scalar.mul(osb[:, kk, 0:D], psA[:, 0:D], rr)
                    # CA = C * A  (GPSIMD, reads the extracted A)
                    nc.gpsimd.tensor_mul(
                        osb[:, kk, D : 2 * D], C1[:, k, 0:D], osb[:, kk, 0:D]
                    )
                    # CBm = (psB * rr) * C  (DVE fused)
                    nc.vector.scalar_tensor_tensor(
                        osb[:, kk, 2 * D : 3 * D], psB, rr, C1[:, k, 0:D], MULT, MULT
                    )
                    if kk == 1:
                        nc.sync.dma_start(
                            out_d.ap()[
                                b, (k - 1) * 128 : (k + 1) * 128, D : 4 * D
                            ].rearrange("(k p) n -> p k n", p=128),
                            osb,
                        )

    nc.compile()
    return nc


def _get_nc():
    if "nc" not in _CACHE:
        _CACHE["nc"] = _build_nc()
    return _CACHE["nc"]


def _make_in_maps(C, Q, cmask, qmask, Wo_w):
    C = np.ascontiguousarray(C, dtype=np.float32)
    Q = np.ascontiguousarray(Q, dtype=np.float32)
    cneg = ((1.0 - cmask.astype(np.float32)) * NEG_INF).astype(np.float32)
    qneg = ((1.0 - qmask.astype(np.float32)) * NEG_INF).astype(np.float32)
    cneg = np.ascontiguousarray(cneg.reshape(B_FULL, KC, 128).transpose(0, 2, 1))
    qneg = np.ascontiguousarray(qneg.reshape(B_FULL, 128, 1))
    Wo_w = Wo_w.astype(np.float32)
    w_pk = np.ascontiguousarray(Wo_w.reshape(6, 128).T)
    w2bc = np.ascontiguousarray(np.broadcast_to(Wo_w[D : 2 * D], (128, D)))
    in_maps = []
    for i in range(N_CORES):
        sl = slice(i * NB, (i + 1) * NB)
        in_maps.append(
            {
                "C": np.ascontiguousarray(C[sl]),
                "Q": np.ascontiguousarray(Q[sl]),
                "cneg": np.ascontiguousarray(cneg[sl]),
                "qneg": np.ascontiguousarray(qneg[sl]),
                "w_pk": w_pk,
                "w2bc": w2bc,
            }
        )
    return in_maps


def kernel(C, Q, cmask, qmask, Wo_w, Wo_b):
    from concourse.bass_utils import run_bass_kernel_spmd

    nc = _get_nc()
    in_maps = _make_in_maps(C, Q, cmask, qmask, Wo_w)
    res = run_bass_kernel_spmd(nc, in_maps, core_ids=list(range(N_CORES)))
    out = np.concatenate([res.results[i]["out"] for i in range(N_CORES)], axis=0)
    return out
